# revision 4
# baseline (speedup 1.0000x reference)
"""Trainium2 Bass kernel for nn_CrossTransformer_36756330119370.

The reference module's attention runs over a single key/value position
(k/v are projections of y reshaped to [B*T, 1, C]), so entmax15 over an
axis of length 1 is identically 1.0 and the q/k projections cancel out
of the forward entirely. The computation reduces exactly (verified
bit-identical on CPU) to:

    w[b, t, :] = Wvo @ y[b, :, t] + bvo        # [C] per (b,t)
    z[b, c, t, v] = x[b, c, t, v] + w[b, t, c]

where Wvo = Wo @ Wv and bvo = Wo @ bv + bo are folded on the host
(standard fusion of two chained linear layers; weights are kernel
constants).

Sharding: data-parallel over B across the 8 NeuronCores (8 batches per
core), folded weights replicated. Per core: one small fp32 matmul on
the PE engine produces w for the core's 960 (b,t) columns; then the
24.6MB x-shard is streamed HBM->SBUF in 1.5MB half-batch tiles, w is
added broadcast over the V axis with a stride-0 access pattern on the
vector engine, and the result streamed back. The kernel is
DMA-fabric-bound (~435 GB/s SBUF AXI ceiling per core).

Queue split: x loads issue on the SP HWDGE ring; the packed-constant
load and all z stores issue on the SWDGE (gpsimd) ring. The SDMA
engines round-robin between the two queues at packet granularity, so a
store waiting on its DVE add never head-of-line-blocks later loads,
and the constant load does not delay the first x tile.
"""

import os
import sys

for _p in ("/opt/trn_rl_repo", "/root/.axon_site/_ro/trn_rl_repo"):
    if os.path.isdir(_p) and _p not in sys.path:
        sys.path.append(_p)

import numpy as np

import concourse.bass as bass
import concourse.mybir as mybir
from concourse.bass_utils import run_bass_kernel_spmd

N_CORES = 8
B, C, T, V = 64, 256, 120, 25
BPC = B // N_CORES          # batches per core
P = 128                     # SBUF partitions
NCC = C // P                # channel chunks (2)
BT = BPC * T                # (b, t) columns per core (960)
NT = 480                    # matmul moving-operand tile (<=512 for fp32)
TV = T * V                  # contiguous elements per (b, c) row (3000)
NHB = 12                    # half-tile pipeline slots (6 full tiles)
NHALF = BPC * NCC           # 16 half-tiles per core

# column offsets inside the packed constant tensor
OFF_W = 0                   # [kc, m] -> kc*C + m          (512 cols)
OFF_B = NCC * C             # 512: [mc]                    (2 cols)
OFF_Y = OFF_B + NCC         # 514: [kc, b, t] -> kc*BT + b*T + t (1920 cols)
PACK_COLS = OFF_Y + NCC * BT  # 2434

FP32 = mybir.dt.float32

# Stash of the last hardware run results (exec_time_ns etc.) for test.py.
LAST_RESULTS = None


def legalize_waits(nc: bass.Bass, max_waits: int = 1) -> None:
    """Split multi-semaphore waits into standalone NoOp wait carriers.

    The walrus build here rejects any instruction carrying more than one
    sync-wait command ("Too many sync wait commands"). A NoOp on the
    same engine stalls the sequencer identically, so hoisting all but
    one wait onto NoOps preserves semantics.
    """
    k = 0
    for blk in nc.m.functions[0].blocks:
        insts = blk.instructions
        i = 0
        while i < len(insts):
            inst = insts[i]
            si = getattr(inst, "sync_info", None)
            if si is not None and si.on_wait and len(si.on_wait) > max_waits:
                waits = list(si.on_wait)
                for w in waits[:-max_waits]:
                    nop = mybir.InstNoOp(name=f"NW-{k}")
                    k += 1
                    nop.engine = inst.engine
                    nop.sync_info = mybir.SyncInfo(on_wait=[w], on_update=[])
                    insts.insert(i, nop)
                    i += 1
                inst.sync_info = mybir.SyncInfo(
                    on_wait=waits[-max_waits:], on_update=si.on_update)
            i += 1


def build_nc_raw() -> bass.Bass:
    """Hand-synchronized raw-bass build (no Tile entry/exit machinery).

    Per-half-slot cumulative counting semaphores: slot j's DMAs
    (load_j -> store_j -> load_{j+12} -> store_{j+12}) are strictly
    serialized by the compute chain, so waits on 16/32/48/64 are
    alias-free even with loads and stores on different queues. Every
    instruction carries at most one sync wait (walrus limit).
    """
    nc = bass.Bass("TRN2", debug=False, num_devices=N_CORES)

    x = nc.dram_tensor("x", [BPC, C, T, V], FP32, kind="ExternalInput").ap()
    cpak = nc.dram_tensor("cpak", [P, PACK_COLS], FP32, kind="ExternalInput").ap()
    z = nc.dram_tensor("z", [BPC, C, T, V], FP32, kind="ExternalOutput").ap()

    cs = nc.alloc_sbuf_tensor("cs", [P, PACK_COLS], FP32).ap()
    w_sb = nc.alloc_sbuf_tensor("w_sb", [P, NCC, BT], FP32).ap()
    xts = [nc.alloc_sbuf_tensor(f"xt{i}", [P, NCC, TV], FP32).ap()
           for i in range(NHB // NCC)]
    ps = [nc.alloc_psum_tensor(f"ps{g}", [P, NT], FP32).ap() for g in range(4)]

    sCP = nc.alloc_semaphore("sCP")
    sHS = [nc.alloc_semaphore(f"sHS{i}") for i in range(NHB)]
    sPE = nc.alloc_semaphore("sPE")
    sACT = nc.alloc_semaphore("sACT")
    sDVE = nc.alloc_semaphore("sDVE")

    def half_sb(j):
        """SBUF destination for half-slot j: [P, TV] region."""
        return xts[j // NCC][:, j % NCC, :]

    def half_sb_v(j):
        """Same region viewed [P, 1, T, V] for the DVE add."""
        ti, ci = j // NCC, j % NCC
        return xts[ti].rearrange("p cc (t v) -> p cc t v", v=V)[
            :, ci:ci + 1, :, :]

    def half_dram(ap, b, h):
        """DRAM [P, TV] view of batch b, channel-half h."""
        return ap[b, h * P:(h + 1) * P].rearrange("p t v -> p (t v)")

    # ---- SP (sync) stream: all x half-tile loads ----
    sync = nc.sync
    for i in range(NHALF):
        b, h = i // NCC, i % NCC
        j = i % NHB
        if i >= NHB:
            # slot reuse: wait for store_{i-NHB} to fully drain
            sync.wait_ge(sHS[j], 32)
        sync.dma_start(half_sb(j), half_dram(x, b, h)).then_inc(sHS[j], 16)

    # ---- PE stream: one folded projection, 4 psum groups ----
    # group order (nch, mc): gates align with the DVE consumption order
    # (batches 0-3 live in nch0 columns, 4-7 in nch1; half h needs mc=h).
    PE_ORDER = [(0, 0), (0, 1), (1, 0), (1, 1)]  # (nch, mc)
    nc.tensor.wait_ge(sCP, 16)
    for g, (nch, mc) in enumerate(PE_ORDER):
        for kc in range(NCC):
            col = OFF_W + kc * C + mc * P
            mm = nc.tensor.matmul(
                ps[g],
                lhsT=cs[:, col:col + P],
                rhs=cs[:, OFF_Y + kc * BT + nch * NT:
                       OFF_Y + kc * BT + (nch + 1) * NT],
                start=(kc == 0), stop=(kc == NCC - 1),
            )
        mm.then_inc(sPE)

    # ---- ACT stream: PSUM->SBUF with per-partition bias ----
    for g, (nch, mc) in enumerate(PE_ORDER):
        nc.scalar.wait_ge(sPE, g + 1)
        nc.scalar.add(
            w_sb[:, mc, nch * NT:(nch + 1) * NT],
            ps[g],
            cs[:, OFF_B + mc:OFF_B + mc + 1],
        ).then_inc(sACT)

    # ---- DVE stream: broadcast adds, one per half-tile ----
    for i in range(NHALF):
        b, h = i // NCC, i % NCC
        j = i % NHB
        # w group needed: (nch = b//4, mc = h) -> index in PE_ORDER
        nc.vector.wait_ge(sACT, (0 if b < 4 else 2) + h + 1)
        nc.vector.wait_ge(sHS[j], 16 if i < NHB else 48)
        w_bc = (
            w_sb[:, h:h + 1, b * T:(b + 1) * T]
            .unsqueeze(3)
            .broadcast_to([P, 1, T, V])
        )
        xv = half_sb_v(j)
        nc.vector.tensor_tensor(
            xv, xv, w_bc, mybir.AluOpType.add
        ).then_inc(sDVE)

    # ---- GPSIMD (Pool/SWDGE) stream: constant load + all z stores ----
    nc.gpsimd.dma_start(cs, cpak).then_inc(sCP, 16)
    for i in range(NHALF):
        b, h = i // NCC, i % NCC
        j = i % NHB
        nc.gpsimd.wait_ge(sDVE, i + 1)
        nc.gpsimd.dma_start(half_dram(z, b, h), half_sb(j)).then_inc(sHS[j], 16)
    # drain: all stores complete before kernel end
    for j in range(NHB):
        nc.gpsimd.wait_ge(sHS[j], 64 if j < NHALF - NHB else 32)

    nc.all_engine_barrier()
    nc.clear_and_free_semaphores([sCP] + sHS + [sPE, sACT, sDVE])

    # Drop Bass's const-AP pool init memsets: this kernel never uses
    # const APs (all biases are real SBUF tensors, scalars are
    # immediates), so the four preamble memsets are dead code.
    for blk in nc.m.functions[0].blocks:
        blk.instructions[:] = [
            i for i in blk.instructions
            if not (type(i).__name__ == "InstMemset"
                    and "const-" in str(i.outs[0]))
        ]

    legalize_waits(nc)
    return nc


def pack_consts(y_shard, Wvo, bvo):
    """Build the [P, PACK_COLS] constant tensor for one core."""
    cpak = np.empty((P, PACK_COLS), np.float32)
    # wt[c_in, c_out] = Wvo[c_out, c_in]; cs[p, kc*C + m] = wt[kc*P+p, m]
    cpak[:, OFF_W:OFF_W + NCC * C] = (
        Wvo.T.reshape(NCC, P, C).transpose(1, 0, 2).reshape(P, NCC * C))
    cpak[:, OFF_B:OFF_B + NCC] = bvo.reshape(NCC, P).T
    # y_sb[p, kc*BT + b*T + t] = y[b, kc*P+p, t]
    cpak[:, OFF_Y:] = (
        y_shard.reshape(BPC, NCC, P, T).transpose(2, 1, 0, 3).reshape(P, NCC * BT))
    return cpak


_NC_CACHE = None


def _get_nc():
    global _NC_CACHE
    if _NC_CACHE is None:
        _NC_CACHE = build_nc_raw()
    return _NC_CACHE


def kernel(x, y, Wq=None, bq=None, Wk=None, bk=None, Wv=None, bv=None,
           Wo=None, bo=None, **_unused):
    global LAST_RESULTS
    x = np.ascontiguousarray(np.asarray(x, dtype=np.float32))
    y = np.asarray(y, dtype=np.float32)
    Wv = np.asarray(Wv, dtype=np.float32)
    bv = np.asarray(bv, dtype=np.float32)
    Wo = np.asarray(Wo, dtype=np.float32)
    bo = np.asarray(bo, dtype=np.float32)
    # fold the two chained linear layers into one
    Wvo = (Wo @ Wv).astype(np.float32)
    bvo = (Wo @ bv + bo).astype(np.float32)

    nc = _get_nc()
    in_maps = []
    for c in range(N_CORES):
        sl = slice(c * BPC, (c + 1) * BPC)
        in_maps.append({
            "x": x[sl],
            "cpak": pack_consts(y[sl], Wvo, bvo),
        })

    res = run_bass_kernel_spmd(
        nc, in_maps, list(range(N_CORES)),
        trace=bool(os.environ.get("KERNEL_PROFILE")),
    )
    LAST_RESULTS = res
    return np.concatenate([res.results[c]["z"] for c in range(N_CORES)], axis=0)


# revision 7
# speedup vs baseline: 1.1352x; 1.1352x over previous
"""Trainium2 Bass kernel for nn_CrossTransformer_36756330119370.

The reference module's attention runs over a single key/value position
(k/v are projections of y reshaped to [B*T, 1, C]), so entmax15 over an
axis of length 1 is identically 1.0 and the q/k projections cancel out
of the forward entirely. The computation reduces exactly (verified
bit-identical on CPU) to:

    w[b, t, :] = Wvo @ y[b, :, t] + bvo        # [C] per (b,t)
    z[b, c, t, v] = x[b, c, t, v] + w[b, t, c]

where Wvo = Wo @ Wv and bvo = Wo @ bv + bo are folded on the host
(standard fusion of two chained linear layers; weights are kernel
constants).

Sharding: data-parallel over B across the 8 NeuronCores (8 batches per
core), folded weights replicated. Per core: one small fp32 matmul on
the PE engine produces w for the core's 960 (b,t) columns; then the
24.6MB x-shard is streamed HBM->SBUF in 1.5MB half-batch tiles, w is
added broadcast over the V axis with a stride-0 access pattern on the
vector engine, and the result streamed back. The kernel is
DMA-fabric-bound (~435 GB/s SBUF AXI ceiling per core).

Queue split: x loads issue on the SP HWDGE ring; the packed-constant
load and all z stores issue on the SWDGE (gpsimd) ring. The SDMA
engines round-robin between the two queues at packet granularity, so a
store waiting on its DVE add never head-of-line-blocks later loads,
and the constant load does not delay the first x tile.
"""

import os
import sys

for _p in ("/opt/trn_rl_repo", "/root/.axon_site/_ro/trn_rl_repo"):
    if os.path.isdir(_p) and _p not in sys.path:
        sys.path.append(_p)

import numpy as np

import concourse.bass as bass
import concourse.mybir as mybir
from concourse.bass_utils import run_bass_kernel_spmd

N_CORES = 8
B, C, T, V = 64, 256, 120, 25
BPC = B // N_CORES          # batches per core
P = 128                     # SBUF partitions
NCC = C // P                # channel chunks (2)
BT = BPC * T                # (b, t) columns per core (960)
NT = 480                    # matmul moving-operand tile (<=512 for fp32)
TV = T * V                  # contiguous elements per (b, c) row (3000)
NHB = 8                     # half-tile pipeline slots (4 full tiles)
NHALF = BPC * NCC           # 16 half-tiles per core

# column offsets inside the packed constant tensor
OFF_W = 0                   # [kc, m] -> kc*C + m          (512 cols)
OFF_B = NCC * C             # 512: [mc]                    (2 cols)
OFF_Y = OFF_B + NCC         # 514: [kc, b, t] -> kc*BT + b*T + t (1920 cols)
PACK_COLS = OFF_Y + NCC * BT  # 2434

FP32 = mybir.dt.float32

# Stash of the last hardware run results (exec_time_ns etc.) for test.py.
LAST_RESULTS = None


def legalize_waits(nc: bass.Bass, max_waits: int = 1) -> None:
    """Split multi-semaphore waits into standalone NoOp wait carriers.

    The walrus build here rejects any instruction carrying more than one
    sync-wait command ("Too many sync wait commands"). A NoOp on the
    same engine stalls the sequencer identically, so hoisting all but
    one wait onto NoOps preserves semantics.
    """
    k = 0
    for blk in nc.m.functions[0].blocks:
        insts = blk.instructions
        i = 0
        while i < len(insts):
            inst = insts[i]
            si = getattr(inst, "sync_info", None)
            if si is not None and si.on_wait and len(si.on_wait) > max_waits:
                waits = list(si.on_wait)
                for w in waits[:-max_waits]:
                    nop = mybir.InstNoOp(name=f"NW-{k}")
                    k += 1
                    nop.engine = inst.engine
                    nop.sync_info = mybir.SyncInfo(on_wait=[w], on_update=[])
                    insts.insert(i, nop)
                    i += 1
                inst.sync_info = mybir.SyncInfo(
                    on_wait=waits[-max_waits:], on_update=si.on_update)
            i += 1


def build_nc_raw() -> bass.Bass:
    """Hand-synchronized raw-bass build (no Tile entry/exit machinery).

    Per-half-slot cumulative counting semaphores: slot j's DMAs
    (load_j -> store_j -> load_{j+12} -> store_{j+12}) are strictly
    serialized by the compute chain, so waits on 16/32/48/64 are
    alias-free even with loads and stores on different queues. Every
    instruction carries at most one sync wait (walrus limit).
    """
    nc = bass.Bass("TRN2", debug=False, num_devices=N_CORES)

    x = nc.dram_tensor("x", [BPC, C, T, V], FP32, kind="ExternalInput").ap()
    cpak = nc.dram_tensor("cpak", [P, PACK_COLS], FP32, kind="ExternalInput").ap()
    z = nc.dram_tensor("z", [BPC, C, T, V], FP32, kind="ExternalOutput").ap()

    cs = nc.alloc_sbuf_tensor("cs", [P, PACK_COLS], FP32).ap()
    w_sb = nc.alloc_sbuf_tensor("w_sb", [P, NCC, BT], FP32).ap()
    xts = [nc.alloc_sbuf_tensor(f"xt{i}", [P, NCC, TV], FP32).ap()
           for i in range(NHB // NCC)]
    ps = [nc.alloc_psum_tensor(f"ps{g}", [P, NT], FP32).ap() for g in range(4)]

    sCP = nc.alloc_semaphore("sCP")
    sHS = [nc.alloc_semaphore(f"sHS{i}") for i in range(NHB)]
    sPE = nc.alloc_semaphore("sPE")
    sACT = nc.alloc_semaphore("sACT")
    sDVE = nc.alloc_semaphore("sDVE")

    def half_sb(j):
        """SBUF destination for half-slot j: [P, TV] region."""
        return xts[j // NCC][:, j % NCC, :]

    def half_sb_v(j):
        """Same region viewed [P, 1, T, V] for the DVE add."""
        ti, ci = j // NCC, j % NCC
        return xts[ti].rearrange("p cc (t v) -> p cc t v", v=V)[
            :, ci:ci + 1, :, :]

    def half_dram(ap, b, h):
        """DRAM [P, TV] view of batch b, channel-half h."""
        return ap[b, h * P:(h + 1) * P].rearrange("p t v -> p (t v)")

    # ---- SP (sync) stream: constant load + all x half-tile loads ----
    sync = nc.sync
    sync.dma_start(cs, cpak).then_inc(sCP, 16)
    for i in range(NHALF):
        b, h = i // NCC, i % NCC
        j = i % NHB
        if i >= NHB:
            # slot reuse: wait for store_{i-NHB} to fully drain
            sync.wait_ge(sHS[j], 32)
        sync.dma_start(half_sb(j), half_dram(x, b, h)).then_inc(sHS[j], 16)

    # ---- PE stream: one folded projection, 4 psum groups ----
    # group order (nch, mc): gates align with the DVE consumption order
    # (batches 0-3 live in nch0 columns, 4-7 in nch1; half h needs mc=h).
    PE_ORDER = [(0, 0), (0, 1), (1, 0), (1, 1)]  # (nch, mc)
    nc.tensor.wait_ge(sCP, 16)
    for g, (nch, mc) in enumerate(PE_ORDER):
        for kc in range(NCC):
            col = OFF_W + kc * C + mc * P
            mm = nc.tensor.matmul(
                ps[g],
                lhsT=cs[:, col:col + P],
                rhs=cs[:, OFF_Y + kc * BT + nch * NT:
                       OFF_Y + kc * BT + (nch + 1) * NT],
                start=(kc == 0), stop=(kc == NCC - 1),
            )
        mm.then_inc(sPE)

    # ---- ACT stream: PSUM->SBUF with per-partition bias ----
    for g, (nch, mc) in enumerate(PE_ORDER):
        nc.scalar.wait_ge(sPE, g + 1)
        nc.scalar.add(
            w_sb[:, mc, nch * NT:(nch + 1) * NT],
            ps[g],
            cs[:, OFF_B + mc:OFF_B + mc + 1],
        ).then_inc(sACT)

    # ---- DVE stream: broadcast adds, one per half-tile ----
    for i in range(NHALF):
        b, h = i // NCC, i % NCC
        j = i % NHB
        # w group needed: (nch = b//4, mc = h) -> index in PE_ORDER
        nc.vector.wait_ge(sACT, (0 if b < 4 else 2) + h + 1)
        nc.vector.wait_ge(sHS[j], 16 if i < NHB else 48)
        w_bc = (
            w_sb[:, h:h + 1, b * T:(b + 1) * T]
            .unsqueeze(3)
            .broadcast_to([P, 1, T, V])
        )
        xv = half_sb_v(j)
        nc.vector.tensor_tensor(
            xv, xv, w_bc, mybir.AluOpType.add
        ).then_inc(sDVE)

    # ---- GPSIMD (Pool/SWDGE) stream: all z stores ----
    for i in range(NHALF):
        b, h = i // NCC, i % NCC
        j = i % NHB
        nc.gpsimd.wait_ge(sDVE, i + 1)
        nc.gpsimd.dma_start(half_dram(z, b, h), half_sb(j)).then_inc(sHS[j], 16)
    # drain: all stores complete before kernel end
    for j in range(NHB):
        nc.gpsimd.wait_ge(sHS[j], 64 if j < NHALF - NHB else 32)

    nc.all_engine_barrier()
    nc.clear_and_free_semaphores([sCP] + sHS + [sPE, sACT, sDVE])

    # Drop Bass's const-AP pool init memsets: this kernel never uses
    # const APs (all biases are real SBUF tensors, scalars are
    # immediates), so the four preamble memsets are dead code.
    for blk in nc.m.functions[0].blocks:
        blk.instructions[:] = [
            i for i in blk.instructions
            if not (type(i).__name__ == "InstMemset"
                    and "const-" in str(i.outs[0]))
        ]

    legalize_waits(nc)
    return nc


def pack_consts(y_shard, Wvo, bvo):
    """Build the [P, PACK_COLS] constant tensor for one core."""
    cpak = np.empty((P, PACK_COLS), np.float32)
    # wt[c_in, c_out] = Wvo[c_out, c_in]; cs[p, kc*C + m] = wt[kc*P+p, m]
    cpak[:, OFF_W:OFF_W + NCC * C] = (
        Wvo.T.reshape(NCC, P, C).transpose(1, 0, 2).reshape(P, NCC * C))
    cpak[:, OFF_B:OFF_B + NCC] = bvo.reshape(NCC, P).T
    # y_sb[p, kc*BT + b*T + t] = y[b, kc*P+p, t]
    cpak[:, OFF_Y:] = (
        y_shard.reshape(BPC, NCC, P, T).transpose(2, 1, 0, 3).reshape(P, NCC * BT))
    return cpak


_NC_CACHE = None


def _get_nc():
    global _NC_CACHE
    if _NC_CACHE is None:
        _NC_CACHE = build_nc_raw()
    return _NC_CACHE


def kernel(x, y, Wq=None, bq=None, Wk=None, bk=None, Wv=None, bv=None,
           Wo=None, bo=None, **_unused):
    global LAST_RESULTS
    x = np.ascontiguousarray(np.asarray(x, dtype=np.float32))
    y = np.asarray(y, dtype=np.float32)
    Wv = np.asarray(Wv, dtype=np.float32)
    bv = np.asarray(bv, dtype=np.float32)
    Wo = np.asarray(Wo, dtype=np.float32)
    bo = np.asarray(bo, dtype=np.float32)
    # fold the two chained linear layers into one
    Wvo = (Wo @ Wv).astype(np.float32)
    bvo = (Wo @ bv + bo).astype(np.float32)

    nc = _get_nc()
    in_maps = []
    for c in range(N_CORES):
        sl = slice(c * BPC, (c + 1) * BPC)
        in_maps.append({
            "x": x[sl],
            "cpak": pack_consts(y[sl], Wvo, bvo),
        })

    res = run_bass_kernel_spmd(
        nc, in_maps, list(range(N_CORES)),
        trace=bool(os.environ.get("KERNEL_PROFILE")),
    )
    LAST_RESULTS = res
    return np.concatenate([res.results[c]["z"] for c in range(N_CORES)], axis=0)
